# revision 1
# baseline (speedup 1.0000x reference)
"""GATv2 gene-graph kernel for 8 Trainium2 NeuronCores (Bass/Tile).

Strategy (data-parallel over batch, per the sharding hint):
- Host: shard batch (B=256 -> 32/core), precompute edge structure as static
  one-hot matrices (edge_index is data, known at trace time).
- Per-gene input linear: PE matmuls, kc-outer, all 64 genes accumulate in 4
  persistent PSUM banks; biases via K=1 matmuls; LeakyReLU(0.01) on ACT.
- GATv2 attention: deduped (dst,src) pairs; z = x_l[src]+x_r[dst] via a static
  one-hot PE matmul; LeakyReLU(0.2) on ACT (casting to bf16); att-dot via DVE
  mul + multi-dim reduce; segment softmax without max-subtraction (logits are
  tiny so exp cannot overflow) with ln(edge-count) folded into the scores to
  handle duplicate edges.
- Aggregation: A^T built per-destination with masked one-hot PE matmuls, then
  dense [g,c']x[g,d] PE matmuls accumulated over heads in PSUM.
- Output MLP: PE matmuls with PE-transposes between layers.
"""
import sys
from contextlib import ExitStack

import numpy as np

sys.path.insert(0, "/opt/trn_rl_repo")

import ml_dtypes  # noqa: E402
import concourse.bass as bass  # noqa: E402
import concourse.tile as tile  # noqa: E402
from concourse import bacc, mybir  # noqa: E402

bf16 = ml_dtypes.bfloat16
F32 = mybir.dt.float32
BF = mybir.dt.bfloat16
AF = mybir.ActivationFunctionType
ALU = mybir.AluOpType

G, B, IN, C, H = 64, 256, 1280, 128, 4
HC = H * C  # 512
KC = IN // 128  # 10
NCORES = 8
BC = B // NCORES  # 32
HID1, HID2 = 512, 128
ZB = 4  # batch elements per z-group (DVE op granularity)


def _prep_edges(edge_index):
    sl = np.arange(G, dtype=np.int64)
    src = np.concatenate([np.asarray(edge_index[0]), sl])
    dst = np.concatenate([np.asarray(edge_index[1]), sl])
    upairs, cnt = np.unique(dst * G + src, return_counts=True)
    pd = (upairs // G).astype(np.int64)
    ps = (upairs % G).astype(np.int64)
    p_real = len(upairs)
    n_chunks = (p_real + 127) // 128
    P = n_chunks * 128
    seg_len = np.bincount(pd, minlength=G)
    seg_off = np.zeros(G, np.int64)
    seg_off[1:] = np.cumsum(seg_len)[:-1]
    lncnt = np.full(P, -1e30, np.float32)
    lncnt[:p_real] = np.log(cnt.astype(np.float64)).astype(np.float32)
    cnt720 = np.zeros(P, np.float32)
    cnt720[:p_real] = (cnt.astype(np.float64) / 720.0).astype(np.float32)
    ps_pad = np.zeros(P, np.int64)
    ps_pad[:p_real] = ps
    pd_pad = np.full(P, G - 1, np.int64)
    pd_pad[:p_real] = pd

    OH = np.zeros((n_chunks, 128, 128), bf16)
    for p in range(P):
        ch, k = p // 128, p % 128
        OH[ch, ps_pad[p], k] = 1
        OH[ch, G + pd_pad[p], k] = 1

    # AT-build plan: per destination d, pieces of its (real) segment per chunk,
    # as zero-masked one-hots [128(pair-in-chunk), 64(g)].
    pieces = []  # (d, ch, start, stop)
    oh_seg = []
    for d in range(G):
        o, l = int(seg_off[d]), int(seg_len[d])
        if l == 0:
            continue  # cannot happen (self loops guarantee l>=1)
        ch_lo, ch_hi = o // 128, (o + l - 1) // 128
        plist = []
        for ch in range(ch_lo, ch_hi + 1):
            lo = max(o, ch * 128)
            hi = min(o + l, (ch + 1) * 128)
            m = np.zeros((128, 64), np.float32)
            for p in range(lo, hi):
                m[p % 128, ps_pad[p]] = 1
            plist.append((ch, m))
        for i, (ch, m) in enumerate(plist):
            pieces.append((d, ch, i == 0, i == len(plist) - 1))
            oh_seg.append(m)
    oh_seg = np.stack(oh_seg)  # [n_pieces, 128, 64]

    # segment bounds on the padded list (pads live in d=63's tail; their
    # lncnt=-1e30 makes them contribute 0 to the denominator)
    seg_bounds = []
    for d in range(G):
        o, l = int(seg_off[d]), int(seg_len[d])
        if d == G - 1:
            l += P - p_real
        seg_bounds.append((o, l))

    return dict(P=P, n_chunks=n_chunks, lncnt=lncnt, cnt720=cnt720, OH=OH,
                oh_seg=oh_seg, pieces=pieces, seg_bounds=seg_bounds)


def _build(E, lrelu_act=True, poly_exp=True):
    P, n_chunks = E["P"], E["n_chunks"]
    pieces = E["pieces"]
    n_pieces = len(pieces)

    nc = bacc.Bacc("TRN2", target_bir_lowering=False, debug=False)

    def din(name, shape, dt=F32):
        return nc.dram_tensor(name, list(shape), dt, kind="ExternalInput").ap()

    peT = din("peT", [KC, 128, G * BC])
    Win = din("Win", [G, IN, C])
    binv = din("binv", [1, G * C])
    onesv = din("onesv", [1, 64])
    Wl = din("Wl", [C, HC])
    Wr = din("Wr", [C, HC])
    blv = din("blv", [1, HC])
    brv = din("brv", [1, HC])
    OHd = din("OH", [n_chunks, 128, 128], BF)
    OHseg = din("OHseg", [n_pieces, 128, 64])
    lncntd = din("lncnt", [128, P])
    cntd = din("cnt720", [128, P])
    attrep = din("attrep", [128, ZB * HC], BF)
    gbias = din("gbias", [C, 1])
    identd = din("ident", [128, 128])
    W1d = din("W1", [G * C, HID1])
    b1v = din("b1v", [1, HID1])
    W2d = din("W2", [HID1, HID2])
    b2v = din("b2v", [1, HID2])
    W3d = din("W3", [HID2, 1])
    outd = nc.dram_tensor("out", [BC, 1], F32, kind="ExternalOutput").ap()
    # DRAM scratch for the fp32 message features (SBUF is tight)
    xlagg_d = nc.dram_tensor("xlaggd", [BC, 64, HC], F32).ap()

    with tile.TileContext(nc) as tc, ExitStack() as ctx:
        pers = ctx.enter_context(tc.tile_pool(name="pers", bufs=1))

        # persistent data tiles
        xT = pers.tile([128, G, BC], F32, tag="xT")
        XLR = pers.tile([128, BC * HC], BF, tag="XLR")
        Sv = pers.tile([128, P], F32, tag="Sv")
        expS = pers.tile([128, P], F32, tag="expS")
        Av = pers.tile([128, P], F32, tag="Av")
        ATs = pers.tile([64, G, 128], F32, tag="ATs")
        M1 = pers.tile([128, BC, G], F32, tag="M1")

        # constants
        ones_t = pers.tile([1, 64], F32, tag="ones")
        nc.sync.dma_start(ones_t[:], onesv)
        binv_t = pers.tile([1, G * C], F32, tag="binv")
        nc.sync.dma_start(binv_t[:], binv)
        Wl_t = pers.tile([C, HC], F32, tag="Wl")
        nc.sync.dma_start(Wl_t[:], Wl)
        Wr_t = pers.tile([C, HC], F32, tag="Wr")
        nc.sync.dma_start(Wr_t[:], Wr)
        blv_t = pers.tile([1, HC], F32, tag="blv")
        nc.sync.dma_start(blv_t[:], blv)
        brv_t = pers.tile([1, HC], F32, tag="brv")
        nc.sync.dma_start(brv_t[:], brv)
        if poly_exp:
            lnc_t = pers.tile([128, P], F32, tag="lnc")
            nc.sync.dma_start(lnc_t[:], cntd)
        else:
            lnc_t = pers.tile([128, P], F32, tag="lnc")
            nc.sync.dma_start(lnc_t[:], lncntd)
        att_t = pers.tile([128, ZB * 4, C], BF, tag="att")
        nc.sync.dma_start(att_t[:], attrep.rearrange("p (g c) -> p g c", c=C))
        gb_t = pers.tile([C, 1], F32, tag="gb")
        nc.sync.dma_start(gb_t[:], gbias)
        id_t = pers.tile([128, 128], F32, tag="id")
        nc.sync.dma_start(id_t[:], identd)
        b1_t = pers.tile([1, HID1], F32, tag="b1")
        nc.sync.dma_start(b1_t[:], b1v)
        W2_t = pers.tile([128, 4, HID2], F32, tag="W2")
        nc.sync.dma_start(W2_t[:], W2d.rearrange("(k p) c -> p k c", p=128))
        b2_t = pers.tile([1, HID2], F32, tag="b2")
        nc.sync.dma_start(b2_t[:], b2v)
        W3_t = pers.tile([HID2, 1], F32, tag="W3")
        nc.sync.dma_start(W3_t[:], W3d)

        def emit_lrelu(out_ap, in_ap, alpha):
            if lrelu_act:
                nc.scalar.activation(out_ap, in_ap, AF.Lrelu, alpha=alpha)
            else:
                nc.vector.scalar_tensor_tensor(out_ap, in_ap, alpha, in_ap,
                                               ALU.mult, ALU.max)

        # ---- Stage A: per-gene input linear ----
        # kc-outer; all 64 genes accumulate in 4 persistent PSUM banks
        # (bank q holds genes 16q..16q+15 as [128c, 16*32b]).
        with tc.tile_pool(name="pep", bufs=2) as pep, \
             tc.tile_pool(name="wp", bufs=6) as wp, \
             tc.tile_pool(name="aps", bufs=1, space="PSUM") as aps:
            xbank = [aps.tile([128, 512], F32, tag=f"xb{q}", name=f"xb{q}")
                     for q in range(4)]
            for kc in range(KC):
                pt = pep.tile([128, G * BC], F32, tag="pe")
                nc.sync.dma_start(pt[:], peT[kc])
                for g in range(G):
                    wt = wp.tile([128, C], F32, tag="wt")
                    nc.sync.dma_start(wt[:], Win[g, kc * 128:(kc + 1) * 128, :])
                    nc.tensor.matmul(
                        xbank[g // 16][:, (g % 16) * BC:(g % 16 + 1) * BC],
                        wt[:], pt[:, g * BC:(g + 1) * BC],
                        start=(kc == 0 and g % 16 == 0), stop=False)
            for g in range(G):
                nc.tensor.matmul(
                    xbank[g // 16][:, (g % 16) * BC:(g % 16 + 1) * BC],
                    binv_t[:, g * C:(g + 1) * C], ones_t[:, :BC],
                    start=False, stop=(g % 16 == 15))
            for q in range(4):
                emit_lrelu(xT[:, q * 16:(q + 1) * 16, :], xbank[q][:], 0.01)

        # ---- Stage B: x_l / x_r transforms ----
        with tc.tile_pool(name="bps", bufs=2, space="PSUM") as bps, \
             tc.tile_pool(name="bstg", bufs=3) as bstg:
            for b in range(BC):
                xsl = xT[:, :, b]  # [128c, 64g]
                psl = bps.tile([64, HC], F32, tag="psl")
                nc.tensor.matmul(psl[:], xsl, Wl_t[:], start=True, stop=False)
                nc.tensor.matmul(psl[:], ones_t[:, :64], blv_t[:],
                                 start=False, stop=True)
                # x@Wl + bl -> fp32 message features (DRAM scratch via SBUF)
                stg = bstg.tile([64, HC], F32, tag="stg")
                nc.vector.tensor_copy(stg[:], psl[:])
                nc.sync.dma_start(xlagg_d[b], stg[:])
                nc.scalar.activation(XLR[0:64, b * HC:(b + 1) * HC], psl[:],
                                     AF.Copy)
                # b_r rides on the R half (attention z is the sum of halves)
                psr = bps.tile([64, HC], F32, tag="psr")
                nc.tensor.matmul(psr[:], xsl, Wr_t[:], start=True, stop=False)
                nc.tensor.matmul(psr[:], ones_t[:, :64], brv_t[:],
                                 start=False, stop=True)
                nc.scalar.activation(XLR[64:128, b * HC:(b + 1) * HC], psr[:],
                                     AF.Copy)

        # ---- Stage C: pair features + scores ----
        with tc.tile_pool(name="ohp", bufs=2) as ohp, \
             tc.tile_pool(name="zps", bufs=3, space="PSUM") as zps, \
             tc.tile_pool(name="zlp", bufs=2) as zlp, \
             tc.tile_pool(name="sap", bufs=2) as sap, \
             tc.tile_pool(name="tps", bufs=2, space="PSUM") as tps:
            for ch in range(n_chunks):
                oht = ohp.tile([128, 128], BF, tag="oh")
                nc.sync.dma_start(oht[:], OHd[ch])
                sat = sap.tile([128, 128], F32, tag="sa")
                for bg in range(BC // ZB):
                    zt = zlp.tile([128, ZB * 4, C], BF, tag="zt")
                    for bi in range(ZB):
                        b = bg * ZB + bi
                        zp = zps.tile([128, HC], F32, tag="zp")
                        nc.tensor.matmul(zp[:], oht[:],
                                         XLR[:, b * HC:(b + 1) * HC],
                                         start=True, stop=True)
                        emit_lrelu(zt[:, bi * 4:(bi + 1) * 4, :], zp[:], 0.2)
                    nc.vector.tensor_mul(zt[:], zt[:], att_t[:])
                    nc.vector.tensor_reduce(
                        sat[:, bg * ZB * 4:(bg + 1) * ZB * 4], zt[:],
                        axis=mybir.AxisListType.X, op=ALU.add)
                tp = tps.tile([128, 128], F32, tag="tp")
                nc.tensor.transpose(tp[:], sat[:], id_t[:])
                if poly_exp:
                    nc.scalar.activation(Sv[:, ch * 128:(ch + 1) * 128],
                                         tp[:], AF.Copy)
                else:
                    nc.vector.tensor_add(Sv[:, ch * 128:(ch + 1) * 128],
                                         tp[:],
                                         lnc_t[:, ch * 128:(ch + 1) * 128])

        # ---- Stage D: segment softmax (no max-sub; logits are tiny) ----
        den_t = pers.tile([128, 64], F32, tag="den")
        rden_t = pers.tile([128, 64], F32, tag="rden")
        if poly_exp:
            # 720*exp(x) ~= (((((x+6)x+30)x+120)x+360)x+720)x+720, |x|<~1
            # then alpha_unnorm = (t+720) * cnt/720  (cnt=0 kills padding)
            pt_ = pers.tile([128, P], F32, tag="polyt")
            nc.vector.scalar_tensor_tensor(pt_[:], Sv[:], 6.0, Sv[:],
                                           ALU.add, ALU.mult)
            for c in (30.0, 120.0, 360.0, 720.0):
                nc.vector.scalar_tensor_tensor(pt_[:], pt_[:], c, Sv[:],
                                               ALU.add, ALU.mult)
            nc.vector.scalar_tensor_tensor(expS[:], pt_[:], 720.0, lnc_t[:],
                                           ALU.add, ALU.mult)
        else:
            nc.scalar.activation(expS[:], Sv[:], AF.Exp)
        seg_bounds = E["seg_bounds"]
        for d in range(G):
            o, l = seg_bounds[d]
            nc.vector.tensor_reduce(den_t[:, d:d + 1], expS[:, o:o + l],
                                    axis=mybir.AxisListType.X, op=ALU.add)
        nc.vector.reciprocal(rden_t[:], den_t[:])
        for d in range(G):
            o, l = seg_bounds[d]
            nc.vector.tensor_scalar_mul(Av[:, o:o + l], expS[:, o:o + l],
                                        rden_t[:, d:d + 1])

        # ---- Stage E: build AT[g, d, bh] ----
        with tc.tile_pool(name="etp", bufs=2, space="PSUM") as etp, \
             tc.tile_pool(name="at1p", bufs=n_chunks) as at1p, \
             tc.tile_pool(name="osp", bufs=3) as osp, \
             tc.tile_pool(name="atp", bufs=2, space="PSUM") as atp:
            at1 = []
            for ch in range(n_chunks):
                tpp = etp.tile([128, 128], F32, tag="etp")
                nc.tensor.transpose(tpp[:], Av[:, ch * 128:(ch + 1) * 128],
                                    id_t[:])
                a1 = at1p.tile([128, 128], F32, tag="at1")
                nc.scalar.activation(a1[:], tpp[:], AF.Copy)
                at1.append(a1)
            cur_ps = None
            for i, (d, ch, st, sp) in enumerate(pieces):
                if st:
                    cur_ps = atp.tile([64, 128], F32, tag="atps")
                ot = osp.tile([128, 64], F32, tag="ohseg")
                nc.sync.dma_start(ot[:], OHseg[i])
                nc.tensor.matmul(cur_ps[:], ot[:], at1[ch][:],
                                 start=st, stop=sp)
                if sp:
                    nc.scalar.activation(ATs[:, d, :], cur_ps[:], AF.Copy)

        # ---- Stage agg: out[c', d] per b, heads accumulated in PSUM ----
        with tc.tile_pool(name="gps", bufs=2, space="PSUM") as gps, \
             tc.tile_pool(name="xlp", bufs=3) as xlp:
            for b in range(BC):
                xlt = xlp.tile([64, HC], F32, tag="xlt")
                nc.sync.dma_start(xlt[:], xlagg_d[b])
                gp = gps.tile([128, G], F32, tag="gp")
                for h in range(H):
                    nc.tensor.matmul(gp[:], xlt[:, h * C:(h + 1) * C],
                                     ATs[:, :, b * H + h],
                                     start=(h == 0), stop=(h == H - 1))
                nc.scalar.activation(M1[:, b, :], gp[:], AF.Identity,
                                     bias=gb_t[:, 0:1], scale=0.25)

        # ---- Stage F: MLP ----
        with tc.tile_pool(name="fps", bufs=1, space="PSUM") as fps, \
             tc.tile_pool(name="fp", bufs=1) as fp, \
             tc.tile_pool(name="w1p", bufs=3) as w1p:
            h1ps = fps.tile([BC, HID1], F32, tag="h1ps")
            for d in range(G):
                w1t = w1p.tile([128, HID1], F32, tag="w1t")
                nc.sync.dma_start(w1t[:], W1d[d * 128:(d + 1) * 128, :])
                nc.tensor.matmul(h1ps[:], M1[:, :, d], w1t[:],
                                 start=(d == 0), stop=False)
            nc.tensor.matmul(h1ps[:], ones_t[:, :BC], b1_t[:],
                             start=False, stop=True)
            h1 = fp.tile([BC, HID1], F32, tag="h1")
            nc.scalar.activation(h1[:], h1ps[:], AF.Relu)
            h1T = fp.tile([128, 4, BC], F32, tag="h1T")
            for k in range(4):
                tp = fps.tile([128, BC], F32, tag="ftp")
                nc.tensor.transpose(tp[:], h1[:, k * 128:(k + 1) * 128],
                                    id_t[0:BC, 0:BC])
                nc.scalar.activation(h1T[:, k, :], tp[:], AF.Copy)
            h2ps = fps.tile([BC, HID2], F32, tag="h2ps")
            for k in range(4):
                nc.tensor.matmul(h2ps[:], h1T[:, k, :], W2_t[:, k, :],
                                 start=(k == 0), stop=False)
            nc.tensor.matmul(h2ps[:], ones_t[:, :BC], b2_t[:],
                             start=False, stop=True)
            h2 = fp.tile([BC, HID2], F32, tag="h2")
            nc.scalar.activation(h2[:], h2ps[:], AF.Relu)
            h2tp = fps.tile([HID2, BC], F32, tag="h2tp")
            nc.tensor.transpose(h2tp[:], h2[:], id_t[0:BC, 0:BC])
            h2T = fp.tile([HID2, BC], F32, tag="h2T")
            nc.scalar.activation(h2T[:], h2tp[:], AF.Copy)
            ops = fps.tile([BC, 1], F32, tag="ops")
            nc.tensor.matmul(ops[:], h2T[:], W3_t[:], start=True, stop=True)
            outs = fp.tile([BC, 1], F32, tag="outs")
            nc.scalar.activation(outs[:], ops[:], AF.Copy)
            nc.sync.dma_start(outd, outs[:])

    nc.compile()
    return nc


def _host_prep(inputs):
    pe = np.asarray(inputs["protein_embeddings"], np.float32)
    E = _prep_edges(np.asarray(inputs["edge_index"]))

    att = np.asarray(inputs["att"], np.float32)  # [H, C]
    attflat = att.reshape(1, HC)
    attrep = np.broadcast_to(attflat, (ZB, HC)).reshape(1, ZB * HC)
    attrep = np.broadcast_to(attrep, (128, ZB * HC)).astype(bf16)

    shared = {
        "Win": np.ascontiguousarray(np.asarray(inputs["W_in"], np.float32)),
        "binv": np.asarray(inputs["b_in"], np.float32).reshape(1, G * C),
        "onesv": np.ones((1, 64), np.float32),
        "Wl": np.ascontiguousarray(np.asarray(inputs["W_l"], np.float32)),
        "Wr": np.ascontiguousarray(np.asarray(inputs["W_r"], np.float32)),
        "blv": np.asarray(inputs["b_l"], np.float32).reshape(1, HC),
        "brv": np.asarray(inputs["b_r"], np.float32).reshape(1, HC),
        "OH": E["OH"],
        "OHseg": E["oh_seg"],
        "lncnt": np.ascontiguousarray(
            np.broadcast_to(E["lncnt"][None, :], (128, E["P"]))),
        "cnt720": np.ascontiguousarray(
            np.broadcast_to(E["cnt720"][None, :], (128, E["P"]))),
        "attrep": np.ascontiguousarray(attrep),
        "gbias": np.asarray(inputs["bias"], np.float32).reshape(C, 1),
        "ident": np.eye(128, dtype=np.float32),
        "W1": np.ascontiguousarray(np.asarray(inputs["W1"], np.float32)),
        "b1v": np.asarray(inputs["b1"], np.float32).reshape(1, HID1),
        "W2": np.ascontiguousarray(np.asarray(inputs["W2"], np.float32)),
        "b2v": np.asarray(inputs["b2"], np.float32).reshape(1, HID2),
        "W3": np.ascontiguousarray(np.asarray(inputs["W3"], np.float32)),
    }
    in_maps = []
    for j in range(NCORES):
        pes = pe[:, j * BC:(j + 1) * BC, :]  # [G, BC, IN]
        peT = np.ascontiguousarray(pes.transpose(2, 0, 1)) \
            .reshape(KC, 128, G * BC)
        m = dict(shared)
        m["peT"] = np.ascontiguousarray(peT)
        in_maps.append(m)
    return E, in_maps


def kernel(**inputs):
    from concourse.bass_utils import run_bass_kernel_spmd
    E, in_maps = _host_prep(inputs)
    nc = _build(E)
    res = run_bass_kernel_spmd(nc, in_maps, list(range(NCORES)))
    b3 = np.asarray(inputs["b3"], np.float32).reshape(1, 1)
    out = np.concatenate([res.results[j]["out"] for j in range(NCORES)],
                         axis=0) + b3
    return out.astype(np.float32)

